# revision 30
# baseline (speedup 1.0000x reference)
"""FNet transformer block kernel for Trainium2 (8 NeuronCores, data-parallel over batch).

Math notes
----------
reference computes, per batch b:
    ft  = Re( FFT_seq( FFT_hid( FFT_hid( x ))))        (hidden FFT applied twice)
    u   = x + ft;  t = LayerNorm(u) * g + beta
    out = (gelu(t @ w1 + b1) @ w2 + b2) * mask

Double FFT along hidden (D=1024):  (F_D^2 x)[d] = D * x[(-d) mod D]  (real).
So with w[t, d] = 1024 * x[t, (-d) mod 1024]:
    ft = Re(F_S) @ w = C @ w,   C[s, t] = cos(2*pi*s*t/2048)   (S=2048)
C is symmetric in both index reflections, so the 2048x2048 cosine transform
folds to a ~1025x1025 one:  zt = wf.T @ Cf with wf the t-folded w; ft.T
columns for s > 1024 mirror zt columns 2048-s.

Everything downstream stays TRANSPOSED (feature axis on partitions, tokens on
the free axis), with weights as the stationary matmul operand:
    FFN1:  psA[j, s] = sum_d w1p[d, j] * v[d, s] + wsum1[j] * c[s]
           v = u * rbc (token-wise LN scale broadcast),  c = -rsqrt_row * S1
    GELU applies b1p[j] as a free per-partition ACT bias; FFN2 consumes H.T
    directly (no PE transposes), b2[j] rides the PSUM->SBUF ACT copy.
Output is produced as out.T in DRAM; the host transposes it back.
Activations and FFN weights are bf16 (1 cycle/row matmuls, 2x DVE, half SBUF);
the FFT runs in fp32r with N=352 column chunks (full-rate, fits a PSUM bank).
"""

import sys
from contextlib import ExitStack

import numpy as np

sys.path.insert(0, "/opt/trn_rl_repo")

import ml_dtypes  # noqa: E402

import concourse.bass as bass  # noqa: E402
import concourse.mybir as mybir  # noqa: E402
import concourse.tile as tile  # noqa: E402
from concourse import bacc  # noqa: E402
from concourse.bass_utils import run_bass_kernel_spmd  # noqa: E402

S, D = 2048, 1024
TF = 1152  # folded-t rows: 1025 padded up to 9*128
SF = 1056  # folded-s cols: 1025 padded up to 1056
NCORES = 8
LN_EPS = 1e-5
EPS_P = float(D) * float(D) * LN_EPS
F32 = mybir.dt.float32
F32R = mybir.dt.float32r
BF16 = mybir.dt.bfloat16
DT = D // 128   # 8
SC = 512        # token chunk width
NSC = S // SC   # 4
# radix-2 even/odd split of the folded cosine transform:
#   E[d,s] = sum_p wf[2p,d]   cos(2pi p s/1024),        p in [0,512], s in [0,512]
#   O[d,s] = sum_p wf[2p+1,d] cos(pi (2p+1) s/1024),    p in [0,511]
#   zt[:, 0:513] = E + O;  zt[:, s'] = E[1024-s'] - O[1024-s'] for s' in (512,1024]
KE, KO = 5, 4        # 513->640 and 512 rows of 128
CEC = 528            # 513 cols padded to 2*264
RCH = [(0, 264), (264, 264)]
BF = ml_dtypes.bfloat16
AF = mybir.ActivationFunctionType


def _r(ap):
    return ap.bitcast(F32R)


def _emit_kernel(ctx: ExitStack, tc: tile.TileContext, xT, wfe, wfo, ce, co,
                 w1b, w2b, wsum1r, b1c, b2c, onesb, onesD, outT):
    nc = tc.nc

    cpool = ctx.enter_context(tc.tile_pool(name="consts", bufs=1))
    ones_col = cpool.tile([128, 1], BF16, tag="ones_col")
    nc.sync.dma_start(ones_col[:], onesb[:])
    onesDi_row = cpool.tile([1, 128], F32R, tag="onesDi_row")
    nc.sync.dma_start(onesDi_row[:], onesD[:])
    eps_t = cpool.tile([1, 1], F32, tag="eps_t")
    nc.gpsimd.memset(eps_t[:], EPS_P)
    wsum1_s = cpool.tile([1, D], BF16, tag="wsum1")
    nc.sync.dma_start(wsum1_s[:], wsum1r[:])
    b1c_s = cpool.tile([128, DT], F32, tag="b1c")
    nc.sync.dma_start(b1c_s[:], b1c[:])
    b2c_s = cpool.tile([128, DT], F32, tag="b2c")
    nc.sync.dma_start(b2c_s[:], b2c[:])

    # FFN weights stay resident (bf16); DMAs issued after the FFT operands
    # so the wf/cf loads win the initial HBM bandwidth race.
    wpool = ctx.enter_context(tc.tile_pool(name="w12", bufs=1))
    w1_s = [wpool.tile([128, D], BF16, tag=f"w1_{dt_}", name=f"w1_{dt_}")
            for dt_ in range(DT)]
    w2_s = [wpool.tile([128, D], BF16, tag=f"w2_{dt_}", name=f"w2_{dt_}")
            for dt_ in range(DT)]

    # zt (folded FFT output), resident through the u-adds
    zpool = ctx.enter_context(tc.tile_pool(name="zt", bufs=1))
    zt_s = [zpool.tile([128, SF], F32, tag=f"zt{m}", name=f"zt{m}")
            for m in range(DT)]

    # ---------------- Phase 1: radix-2 folded cosine transform ----------
    with tc.tile_pool(name="fft_in", bufs=1) as fpool, \
         tc.tile_pool(name="osb", bufs=3) as opool_o, \
         tc.tile_pool(name="fpse", bufs=2, space="PSUM") as fpsE, \
         tc.tile_pool(name="fpso", bufs=2, space="PSUM") as fpsO:
        wfe_s, ce_s, wfo_s, co_s = [], [], [], []
        for kt in range(KE):
            a = fpool.tile([128, D], BF16, tag=f"wfe{kt}")
            nc.sync.dma_start(a[:], wfe[kt * 128:(kt + 1) * 128, :])
            wfe_s.append(a)
            c = fpool.tile([128, CEC], BF16, tag=f"ce{kt}")
            nc.sync.dma_start(c[:], ce[kt * 128:(kt + 1) * 128, :])
            ce_s.append(c)
        for kt in range(KO):
            a = fpool.tile([128, D], BF16, tag=f"wfo{kt}")
            nc.sync.dma_start(a[:], wfo[kt * 128:(kt + 1) * 128, :])
            wfo_s.append(a)
            c = fpool.tile([128, CEC], BF16, tag=f"co{kt}")
            nc.sync.dma_start(c[:], co[kt * 128:(kt + 1) * 128, :])
            co_s.append(c)
        for dt_ in range(DT):
            nc.sync.dma_start(w1_s[dt_][:], w1b[dt_ * 128:(dt_ + 1) * 128, :])
            nc.sync.dma_start(w2_s[dt_][:], w2b[dt_ * 128:(dt_ + 1) * 128, :])
        for mt in range(DT):
            msl = slice(mt * 128, (mt + 1) * 128)
            z = zt_s[mt]
            for ci, (n0, nw) in enumerate(RCH):
                pse = fpsE.tile([128, 264], F32, tag="pse")
                for kt in range(KE):
                    nc.tensor.matmul(pse[:], wfe_s[kt][:, msl],
                                     ce_s[kt][:, n0:n0 + nw],
                                     start=(kt == 0), stop=(kt == KE - 1))
                pso = fpsO.tile([128, 264], F32, tag="pso")
                for kt in range(KO):
                    nc.tensor.matmul(pso[:], wfo_s[kt][:, msl],
                                     co_s[kt][:, n0:n0 + nw],
                                     start=(kt == 0), stop=(kt == KO - 1))
                # DVE reads at most one PSUM operand; stage O via ACT copy
                osb = opool_o.tile([128, 264], F32, tag="osb")
                nc.scalar.copy(osb[:], pso[:])
                if ci == 0:
                    nc.vector.tensor_add(z[:, 0:264], pse[:], osb[:])
                    # reversed write: cols 1024..761 <- locals 0..263
                    nc.vector.tensor_sub(z[:, 1024:760:-1], pse[:], osb[:])
                else:
                    nc.vector.tensor_add(z[:, 264:513],
                                         pse[:, 0:249], osb[:, 0:249])
                    # cols 760..513 <- locals 0..247
                    nc.vector.tensor_sub(z[:, 760:512:-1],
                                         pse[:, 0:248], osb[:, 0:248])

    # ---------------- Phase 2: residual + LN + FFN, fully transposed ----
    xpool = ctx.enter_context(tc.tile_pool(name="xs", bufs=4))
    upool = ctx.enter_context(tc.tile_pool(name="u", bufs=1))
    u_s = [upool.tile([128, S], BF16, tag=f"u{d}", name=f"u{d}")
           for d in range(DT)]
    vpool = ctx.enter_context(tc.tile_pool(name="v", bufs=1))
    v_s = [vpool.tile([128, S], BF16, tag=f"v{d}", name=f"v{d}")
           for d in range(DT)]
    hpool = ctx.enter_context(tc.tile_pool(name="h", bufs=1))
    h_s = [hpool.tile([128, S], BF16, tag=f"h{j}", name=f"h{j}")
           for j in range(DT)]
    usqpool = ctx.enter_context(tc.tile_pool(name="usq", bufs=8))
    rowpool = ctx.enter_context(tc.tile_pool(name="rows", bufs=1))
    rbpool = ctx.enter_context(tc.tile_pool(name="rb", bufs=2))
    opool = ctx.enter_context(tc.tile_pool(name="o", bufs=3))
    s1ps = ctx.enter_context(tc.tile_pool(name="s1ps", bufs=1, space="PSUM"))
    s2ps = ctx.enter_context(tc.tile_pool(name="s2ps", bufs=1, space="PSUM"))
    rbps = ctx.enter_context(tc.tile_pool(name="rbps", bufs=1, space="PSUM"))
    aps = ctx.enter_context(tc.tile_pool(name="aps", bufs=2, space="PSUM"))
    ops_ = ctx.enter_context(tc.tile_pool(name="ops", bufs=2, space="PSUM"))

    def emit_prep(sc):
        """DVE: u = x + mirrored zt (bf16 out); usq = u*u (bf16)."""
        s0 = sc * SC
        for d in range(DT):
            xt = xpool.tile([128, SC], F32, tag="xt")
            nc.gpsimd.dma_start(xt[:], xT[d * 128:(d + 1) * 128, s0:s0 + SC])
            z = zt_s[d]
            u = u_s[d]
            if sc <= 1:
                nc.vector.tensor_add(u[:, s0:s0 + SC], xt[:], z[:, s0:s0 + SC])
            elif sc == 2:
                nc.vector.tensor_add(u[:, 1024:1025], xt[:, 0:1],
                                     z[:, 1024:1025])
                nc.vector.tensor_add(u[:, 1025:1536], xt[:, 1:SC],
                                     z[:, 1023:512:-1])
            else:
                nc.vector.tensor_add(u[:, 1536:2048], xt[:], z[:, 512:0:-1])
        usq = []
        for d in range(DT):
            q = usqpool.tile([128, SC], BF16, tag="usq")
            nc.vector.tensor_mul(q[:], u_s[d][:, s0:s0 + SC],
                                 u_s[d][:, s0:s0 + SC])
            usq.append(q)
        return usq

    def emit_stats(sc, usq):
        """PE token stats + LN rows + rbc broadcast + v tiles."""
        s0 = sc * SC
        s1 = s1ps.tile([1, SC], F32, tag="s1")
        for d in range(DT):
            nc.tensor.matmul(s1[0:1, :], ones_col[:], u_s[d][:, s0:s0 + SC],
                             start=(d == 0), stop=(d == DT - 1))
        s2 = s2ps.tile([1, SC], F32, tag="s2")
        for d in range(DT):
            nc.tensor.matmul(s2[0:1, :], ones_col[:], usq[d][:],
                             start=(d == 0), stop=(d == DT - 1))
        s1r = rowpool.tile([1, SC], F32, tag="s1r")
        nc.vector.tensor_copy(s1r[:], s1[0:1, :])
        s2r = rowpool.tile([1, SC], F32, tag="s2r")
        nc.vector.tensor_copy(s2r[:], s2[0:1, :])
        sq = rowpool.tile([1, SC], F32, tag="sq")
        nc.vector.tensor_mul(sq[:], s1r[:], s1r[:])
        vr = rowpool.tile([1, SC], F32, tag="vr")
        # vr = D*S2 - S1^2  (= D^2 * var)
        nc.vector.scalar_tensor_tensor(
            out=vr[:], in0=s2r[:], scalar=float(D), in1=sq[:],
            op0=mybir.AluOpType.mult, op1=mybir.AluOpType.subtract)
        svr = rowpool.tile([1, SC], F32R, tag="svr")
        nc.scalar.activation(svr[:], vr[:], AF.Sqrt,
                             bias=eps_t[0:1, 0:1], scale=1.0)
        # pbs = (1/D) * ones x svr  ->  full-tile broadcast of svr/D, so the
        # reciprocal runs partition-parallel (a [1,512] row reciprocal is
        # serial on one DVE lane and costs ~4us).
        pbs = rbps.tile([128, SC], F32, tag="pbs")
        nc.tensor.matmul(pbs[:], onesDi_row[:], svr[:], start=True, stop=True)
        rb32 = rbpool.tile([128, SC], F32, tag="rb32")
        # = D/svr = 1/sqrt(var+eps); ~18 correct bits, 5x faster than exact
        nc.vector.reciprocal_approx_fast(rb32[:], pbs[:])
        rb = rbpool.tile([128, SC], BF16, tag="rb")
        nc.vector.tensor_copy(rb[:], rb32[:])
        crow = rowpool.tile([1, SC], BF16, tag="crow", bufs=2)
        # crow = -(S1/D) * r_true = -mu * r
        nc.vector.scalar_tensor_tensor(
            out=crow[:], in0=s1r[:], scalar=-1.0 / float(D),
            in1=rb32[0:1, :],
            op0=mybir.AluOpType.mult, op1=mybir.AluOpType.mult)
        for d in range(DT):
            nc.vector.tensor_mul(v_s[d][:, s0:s0 + SC],
                                 u_s[d][:, s0:s0 + SC], rb[:])
        return crow

    def emit_f1(sc, crow):
        s0 = sc * SC
        for j in range(DT):
            pa = aps.tile([128, SC], F32, tag="pa")
            for d in range(DT):
                nc.tensor.matmul(pa[:], w1_s[d][:, j * 128:(j + 1) * 128],
                                 v_s[d][:, s0:s0 + SC],
                                 start=(d == 0), stop=False)
            nc.tensor.matmul(pa[:], wsum1_s[0:1, j * 128:(j + 1) * 128],
                             crow[:], start=False, stop=True)
            nc.scalar.activation(h_s[j][:, s0:s0 + SC], pa[:], AF.Gelu,
                                 bias=b1c_s[:, j:j + 1], scale=1.0)

    def emit_f2(sc):
        s0 = sc * SC
        for do in range(DT):
            po = ops_.tile([128, SC], F32, tag="po")
            for j in range(DT):
                nc.tensor.matmul(po[:], w2_s[j][:, do * 128:(do + 1) * 128],
                                 h_s[j][:, s0:s0 + SC],
                                 start=(j == 0), stop=(j == DT - 1))
            o = opool.tile([128, SC], F32, tag="o")
            nc.scalar.activation(o[:], po[:], AF.Identity,
                                 bias=b2c_s[:, do:do + 1], scale=1.0)
            nc.gpsimd.dma_start(outT[do * 128:(do + 1) * 128, s0:s0 + SC], o[:])

    # Interleaved schedule: PE always has FFN work queued behind each stats
    # segment so the DVE/ACT LN chain latency is hidden.
    usq0 = emit_prep(0)
    c0 = emit_stats(0, usq0)
    usq1 = emit_prep(1)
    emit_f1(0, c0)
    c1 = emit_stats(1, usq1)
    usq2 = emit_prep(2)
    emit_f1(1, c1)
    c2 = emit_stats(2, usq2)
    emit_f2(0)
    usq3 = emit_prep(3)
    emit_f1(2, c2)
    c3 = emit_stats(3, usq3)
    emit_f2(1)
    emit_f1(3, c3)
    emit_f2(2)
    emit_f2(3)


_NC_CACHE = {}


def _build_nc():
    if "nc" in _NC_CACHE:
        return _NC_CACHE["nc"]
    nc = bacc.Bacc("TRN2", target_bir_lowering=False, debug=False)
    xT = nc.declare_dram_parameter("xT", [D, S], F32, isOutput=False)
    wfe = nc.declare_dram_parameter("wfe", [KE * 128, D], BF16, isOutput=False)
    wfo = nc.declare_dram_parameter("wfo", [KO * 128, D], BF16, isOutput=False)
    ce = nc.declare_dram_parameter("ce", [KE * 128, CEC], BF16, isOutput=False)
    co = nc.declare_dram_parameter("co", [KO * 128, CEC], BF16, isOutput=False)
    w1b = nc.declare_dram_parameter("w1b", [D, D], BF16, isOutput=False)
    w2b = nc.declare_dram_parameter("w2b", [D, D], BF16, isOutput=False)
    wsum1r = nc.declare_dram_parameter("wsum1r", [1, D], BF16, isOutput=False)
    b1c = nc.declare_dram_parameter("b1c", [128, DT], F32, isOutput=False)
    b2c = nc.declare_dram_parameter("b2c", [128, DT], F32, isOutput=False)
    onesb = nc.declare_dram_parameter("onesb", [128, 1], BF16, isOutput=False)
    onesD = nc.declare_dram_parameter("onesD", [1, 128], F32R, isOutput=False)
    outT = nc.declare_dram_parameter("outT", [D, S], F32, isOutput=True)
    with tile.TileContext(nc) as tc:
        with ExitStack() as ctx:
            _emit_kernel(ctx, tc, xT, wfe, wfo, ce, co, w1b, w2b, wsum1r,
                         b1c, b2c, onesb, onesD, outT)
    nc.compile()
    _NC_CACHE["nc"] = nc
    return nc


def _host_prep(x, ln_g, ln_b, w1, b1, w2, b2):
    """Build per-core and shared device inputs."""
    B = x.shape[0]
    p_e = np.arange(513, dtype=np.float64)
    p_o = np.arange(512, dtype=np.float64)
    ss = np.arange(513, dtype=np.float64)
    ce_ = np.zeros((KE * 128, CEC), BF)
    ce_[:513, :513] = np.cos(2.0 * np.pi * np.outer(p_e, ss) / 1024.0).astype(BF)
    co_ = np.zeros((KO * 128, CEC), BF)
    co_[:512, :513] = np.cos(
        np.pi * np.outer(2.0 * p_o + 1.0, ss) / 1024.0).astype(BF)

    w1p = (w1 * ln_g[:, None]).astype(np.float32)
    w1pb = w1p.astype(BF)
    w2b_ = np.asarray(w2, np.float32).astype(BF)
    wsum1 = w1pb.astype(np.float64).sum(axis=0).astype(BF).reshape(1, D)
    b1p = (b1 + ln_b @ w1).astype(np.float32)
    b1c_ = np.ascontiguousarray(b1p.reshape(DT, 128).T)
    b2c_ = np.ascontiguousarray(np.asarray(b2, np.float32).reshape(DT, 128).T)

    rev = np.concatenate([[0], np.arange(D - 1, 0, -1)])
    shared = dict(ce=ce_, co=co_, w1b=w1pb, w2b=w2b_, wsum1r=wsum1,
                  b1c=b1c_, b2c=b2c_,
                  onesb=np.ones((128, 1), BF),
                  onesD=np.full((1, 128), 1.0 / float(D), np.float32))

    in_maps = []
    for b in range(B):
        xb = np.asarray(x[b], np.float32)
        w = np.float32(D) * xb[:, rev]
        wf_ = np.zeros((1025, D), np.float32)
        wf_[0] = w[0]
        wf_[1024] = w[1024]
        wf_[1:1024] = w[1:1024] + w[2047:1024:-1]
        wfe_ = np.zeros((KE * 128, D), BF)
        wfe_[:513] = wf_[0::2].astype(BF)
        wfo_ = np.zeros((KO * 128, D), BF)
        wfo_[:512] = wf_[1::2].astype(BF)
        xT = np.ascontiguousarray(xb.T)
        in_maps.append(dict(xT=xT, wfe=wfe_, wfo=wfo_, **shared))
    return in_maps


def _run(inputs, trace=False, trace_kwargs=None):
    x = np.asarray(inputs["x"], np.float32)
    in_maps = _host_prep(
        x,
        np.asarray(inputs["ln_g"], np.float32),
        np.asarray(inputs["ln_b"], np.float32),
        np.asarray(inputs["w1"], np.float32),
        np.asarray(inputs["b1"], np.float32),
        np.asarray(inputs["w2"], np.float32),
        np.asarray(inputs["b2"], np.float32),
    )
    nc = _build_nc()
    res = run_bass_kernel_spmd(nc, in_maps, list(range(NCORES)), trace=trace,
                               **(trace_kwargs or {}))
    outs = np.stack([np.ascontiguousarray(
        np.asarray(res.results[b]["outT"], np.float32).T)
        for b in range(NCORES)])
    outs = outs * np.asarray(inputs["mask"], np.float32)
    return outs, res


def kernel(**inputs) -> np.ndarray:
    out, _ = _run(inputs, trace=False)
    return out


# revision 33
# speedup vs baseline: 1.1829x; 1.1829x over previous
"""FNet transformer block kernel for Trainium2 (8 NeuronCores, data-parallel over batch).

Math notes
----------
reference computes, per batch b:
    ft  = Re( FFT_seq( FFT_hid( FFT_hid( x ))))        (hidden FFT applied twice)
    u   = x + ft;  t = LayerNorm(u) * g + beta
    out = (gelu(t @ w1 + b1) @ w2 + b2) * mask

Double FFT along hidden (D=1024):  (F_D^2 x)[d] = D * x[(-d) mod D]  (real).
So with w[t, d] = 1024 * x[t, (-d) mod 1024]:
    ft = Re(F_S) @ w = C @ w,   C[s, t] = cos(2*pi*s*t/2048)   (S=2048)
C is symmetric in both index reflections, so the 2048x2048 cosine transform
folds to a ~1025x1025 one:  zt = wf.T @ Cf with wf the t-folded w; ft.T
columns for s > 1024 mirror zt columns 2048-s.

Everything downstream stays TRANSPOSED (feature axis on partitions, tokens on
the free axis), with weights as the stationary matmul operand:
    FFN1:  psA[j, s] = sum_d w1p[d, j] * v[d, s] + wsum1[j] * c[s]
           v = u * rbc (token-wise LN scale broadcast),  c = -rsqrt_row * S1
    GELU applies b1p[j] as a free per-partition ACT bias; FFN2 consumes H.T
    directly (no PE transposes), b2[j] rides the PSUM->SBUF ACT copy.
Output is produced as out.T in DRAM; the host transposes it back.
Activations and FFN weights are bf16 (1 cycle/row matmuls, 2x DVE, half SBUF);
the FFT runs in fp32r with N=352 column chunks (full-rate, fits a PSUM bank).
"""

import sys
from contextlib import ExitStack

import numpy as np

sys.path.insert(0, "/opt/trn_rl_repo")

import ml_dtypes  # noqa: E402

import concourse.bass as bass  # noqa: E402
import concourse.mybir as mybir  # noqa: E402
import concourse.tile as tile  # noqa: E402
from concourse import bacc  # noqa: E402
from concourse.bass_utils import run_bass_kernel_spmd  # noqa: E402

S, D = 2048, 1024
TF = 1152  # folded-t rows: 1025 padded up to 9*128
SF = 1056  # folded-s cols: 1025 padded up to 1056
NCORES = 8
LN_EPS = 1e-5
EPS_P = float(D) * float(D) * LN_EPS
F32 = mybir.dt.float32
F32R = mybir.dt.float32r
BF16 = mybir.dt.bfloat16
DT = D // 128   # 8
SC = 512        # token chunk width
NSC = S // SC   # 4
# radix-2 even/odd split of the folded cosine transform:
#   E[d,s] = sum_p wf[2p,d]   cos(2pi p s/1024),        p in [0,512], s in [0,512]
#   O[d,s] = sum_p wf[2p+1,d] cos(pi (2p+1) s/1024),    p in [0,511]
#   zt[:, 0:513] = E + O;  zt[:, s'] = E[1024-s'] - O[1024-s'] for s' in (512,1024]
KE, KO = 5, 4        # 513->640 and 512 rows of 128
CEC = 528            # 513 cols padded to 2*264
RCH = [(0, 264), (264, 264)]
BF = ml_dtypes.bfloat16
AF = mybir.ActivationFunctionType


def _r(ap):
    return ap.bitcast(F32R)


def _emit_kernel(ctx: ExitStack, tc: tile.TileContext, xT, wfe, wfo, ce, co,
                 w1b, w2b, wsum1r, b1c, b2c, onesb, onesD, outT):
    nc = tc.nc

    cpool = ctx.enter_context(tc.tile_pool(name="consts", bufs=1))
    ones_col = cpool.tile([128, 1], BF16, tag="ones_col")
    nc.sync.dma_start(ones_col[:], onesb[:])
    onesDi_row = cpool.tile([1, 128], F32R, tag="onesDi_row")
    nc.sync.dma_start(onesDi_row[:], onesD[:])
    eps_t = cpool.tile([1, 1], F32, tag="eps_t")
    nc.gpsimd.memset(eps_t[:], EPS_P)
    wsum1_s = cpool.tile([1, D], BF16, tag="wsum1")
    nc.sync.dma_start(wsum1_s[:], wsum1r[:])
    b1c_s = cpool.tile([128, DT], F32, tag="b1c")
    nc.sync.dma_start(b1c_s[:], b1c[:])
    b2c_s = cpool.tile([128, DT], F32, tag="b2c")
    nc.sync.dma_start(b2c_s[:], b2c[:])

    # FFN weights stay resident (bf16); DMAs issued after the FFT operands
    # so the wf/cf loads win the initial HBM bandwidth race.
    wpool = ctx.enter_context(tc.tile_pool(name="w12", bufs=1))
    w1_s = [wpool.tile([128, D], BF16, tag=f"w1_{dt_}", name=f"w1_{dt_}")
            for dt_ in range(DT)]
    w2_s = [wpool.tile([128, D], BF16, tag=f"w2_{dt_}", name=f"w2_{dt_}")
            for dt_ in range(DT)]

    # zt (folded FFT output), resident through the u-adds
    zpool = ctx.enter_context(tc.tile_pool(name="zt", bufs=1))
    zt_s = [zpool.tile([128, SF], F32, tag=f"zt{m}", name=f"zt{m}")
            for m in range(DT)]

    # ---------------- Phase 1: radix-2 folded cosine transform ----------
    with tc.tile_pool(name="fft_in", bufs=1) as fpool, \
         tc.tile_pool(name="osb", bufs=3) as opool_o, \
         tc.tile_pool(name="fpse", bufs=2, space="PSUM") as fpsE, \
         tc.tile_pool(name="fpso", bufs=2, space="PSUM") as fpsO:
        wfe_s, ce_s, wfo_s, co_s = [], [], [], []
        for kt in range(KE):
            a = fpool.tile([128, D], BF16, tag=f"wfe{kt}")
            nc.sync.dma_start(a[:], wfe[kt * 128:(kt + 1) * 128, :])
            wfe_s.append(a)
            c = fpool.tile([128, CEC], BF16, tag=f"ce{kt}")
            nc.sync.dma_start(c[:], ce[kt * 128:(kt + 1) * 128, :])
            ce_s.append(c)
        for kt in range(KO):
            a = fpool.tile([128, D], BF16, tag=f"wfo{kt}")
            nc.sync.dma_start(a[:], wfo[kt * 128:(kt + 1) * 128, :])
            wfo_s.append(a)
            c = fpool.tile([128, CEC], BF16, tag=f"co{kt}")
            nc.sync.dma_start(c[:], co[kt * 128:(kt + 1) * 128, :])
            co_s.append(c)
        for dt_ in range(DT):
            nc.sync.dma_start(w1_s[dt_][:], w1b[dt_ * 128:(dt_ + 1) * 128, :])
            nc.sync.dma_start(w2_s[dt_][:], w2b[dt_ * 128:(dt_ + 1) * 128, :])
        for mt in range(DT):
            msl = slice(mt * 128, (mt + 1) * 128)
            z = zt_s[mt]
            for ci, (n0, nw) in enumerate(RCH):
                pse = fpsE.tile([128, 264], F32, tag="pse")
                for kt in range(KE):
                    nc.tensor.matmul(pse[:], wfe_s[kt][:, msl],
                                     ce_s[kt][:, n0:n0 + nw],
                                     start=(kt == 0), stop=(kt == KE - 1))
                pso = fpsO.tile([128, 264], F32, tag="pso")
                for kt in range(KO):
                    nc.tensor.matmul(pso[:], wfo_s[kt][:, msl],
                                     co_s[kt][:, n0:n0 + nw],
                                     start=(kt == 0), stop=(kt == KO - 1))
                # DVE reads at most one PSUM operand; stage O via ACT copy
                osb = opool_o.tile([128, 264], F32, tag="osb")
                nc.scalar.copy(osb[:], pso[:])
                if ci == 0:
                    nc.vector.tensor_add(z[:, 0:264], pse[:], osb[:])
                    # reversed write: cols 1024..761 <- locals 0..263
                    nc.vector.tensor_sub(z[:, 1024:760:-1], pse[:], osb[:])
                else:
                    nc.vector.tensor_add(z[:, 264:513],
                                         pse[:, 0:249], osb[:, 0:249])
                    # cols 760..513 <- locals 0..247
                    nc.vector.tensor_sub(z[:, 760:512:-1],
                                         pse[:, 0:248], osb[:, 0:248])

    # ---------------- Phase 2: residual + LN + FFN, fully transposed ----
    xpool = ctx.enter_context(tc.tile_pool(name="xs", bufs=4))
    upool = ctx.enter_context(tc.tile_pool(name="u", bufs=1))
    u_s = [upool.tile([128, S], BF16, tag=f"u{d}", name=f"u{d}")
           for d in range(DT)]
    vpool = ctx.enter_context(tc.tile_pool(name="v", bufs=1))
    v_s = [vpool.tile([128, S], BF16, tag=f"v{d}", name=f"v{d}")
           for d in range(DT)]
    hpool = ctx.enter_context(tc.tile_pool(name="h", bufs=1))
    h_s = [hpool.tile([128, S], BF16, tag=f"h{j}", name=f"h{j}")
           for j in range(DT)]
    usqpool = ctx.enter_context(tc.tile_pool(name="usq", bufs=8))
    rowpool = ctx.enter_context(tc.tile_pool(name="rows", bufs=1))
    rbpool = ctx.enter_context(tc.tile_pool(name="rb", bufs=2))
    opool = ctx.enter_context(tc.tile_pool(name="o", bufs=3))
    s1ps = ctx.enter_context(tc.tile_pool(name="s1ps", bufs=1, space="PSUM"))
    s2ps = ctx.enter_context(tc.tile_pool(name="s2ps", bufs=1, space="PSUM"))
    rbps = ctx.enter_context(tc.tile_pool(name="rbps", bufs=1, space="PSUM"))
    aps = ctx.enter_context(tc.tile_pool(name="aps", bufs=2, space="PSUM"))
    ops_ = ctx.enter_context(tc.tile_pool(name="ops", bufs=2, space="PSUM"))

    def emit_prep(sc):
        """DVE: u = x + mirrored zt (bf16 out); usq = u*u (bf16)."""
        s0 = sc * SC
        for d in range(DT):
            xt = xpool.tile([128, SC], F32, tag="xt")
            nc.gpsimd.dma_start(xt[:], xT[d * 128:(d + 1) * 128, s0:s0 + SC])
            z = zt_s[d]
            u = u_s[d]
            if sc <= 1:
                nc.vector.tensor_add(u[:, s0:s0 + SC], xt[:], z[:, s0:s0 + SC])
            elif sc == 2:
                nc.vector.tensor_add(u[:, 1024:1025], xt[:, 0:1],
                                     z[:, 1024:1025])
                nc.vector.tensor_add(u[:, 1025:1536], xt[:, 1:SC],
                                     z[:, 1023:512:-1])
            else:
                nc.vector.tensor_add(u[:, 1536:2048], xt[:], z[:, 512:0:-1])
        usq = []
        for d in range(DT):
            q = usqpool.tile([128, SC], BF16, tag="usq")
            nc.vector.tensor_mul(q[:], u_s[d][:, s0:s0 + SC],
                                 u_s[d][:, s0:s0 + SC])
            usq.append(q)
        return usq

    def emit_stats(sc, usq):
        """PE token stats + LN rows + rbc broadcast + v tiles."""
        s0 = sc * SC
        s1 = s1ps.tile([1, SC], F32, tag="s1")
        for d in range(DT):
            nc.tensor.matmul(s1[0:1, :], ones_col[:], u_s[d][:, s0:s0 + SC],
                             start=(d == 0), stop=(d == DT - 1))
        s2 = s2ps.tile([1, SC], F32, tag="s2")
        for d in range(DT):
            nc.tensor.matmul(s2[0:1, :], ones_col[:], usq[d][:],
                             start=(d == 0), stop=(d == DT - 1))
        s1r = rowpool.tile([1, SC], F32, tag="s1r")
        nc.vector.tensor_copy(s1r[:], s1[0:1, :])
        s2r = rowpool.tile([1, SC], F32, tag="s2r")
        nc.scalar.copy(s2r[:], s2[0:1, :])  # on ACT, parallel with s1r
        sq = rowpool.tile([1, SC], F32, tag="sq")
        nc.vector.tensor_mul(sq[:], s1r[:], s1r[:])
        vr = rowpool.tile([1, SC], F32, tag="vr")
        # vr = D*S2 - S1^2  (= D^2 * var)
        nc.vector.scalar_tensor_tensor(
            out=vr[:], in0=s2r[:], scalar=float(D), in1=sq[:],
            op0=mybir.AluOpType.mult, op1=mybir.AluOpType.subtract)
        # rsq = 1/sqrt(vr + D^2*eps)  (vr+eps > 0, so abs is a no-op)
        rsq = rowpool.tile([1, SC], F32R, tag="rsq")
        nc.scalar.activation(rsq[:], vr[:], AF.Abs_reciprocal_sqrt,
                             bias=eps_t[0:1, 0:1], scale=1.0)
        # pbs = (D * ones) x rsq = r_true broadcast to all partitions
        pbs = rbps.tile([128, SC], F32, tag="pbs")
        nc.tensor.matmul(pbs[:], onesDi_row[:], rsq[:], start=True, stop=True)
        rb = rbpool.tile([128, SC], BF16, tag="rb")
        nc.vector.tensor_copy(rb[:], pbs[:])
        crow = rowpool.tile([1, SC], BF16, tag="crow", bufs=2)
        # crow = -(S1/D) * r_true = -mu * r
        nc.vector.scalar_tensor_tensor(
            out=crow[:], in0=s1r[:], scalar=-1.0 / float(D),
            in1=pbs[0:1, :],
            op0=mybir.AluOpType.mult, op1=mybir.AluOpType.mult)
        for d in range(DT):
            nc.vector.tensor_mul(v_s[d][:, s0:s0 + SC],
                                 u_s[d][:, s0:s0 + SC], rb[:])
        return crow

    def emit_f1(sc, crow):
        s0 = sc * SC
        for j in range(DT):
            pa = aps.tile([128, SC], F32, tag="pa")
            for d in range(DT):
                nc.tensor.matmul(pa[:], w1_s[d][:, j * 128:(j + 1) * 128],
                                 v_s[d][:, s0:s0 + SC],
                                 start=(d == 0), stop=False)
            nc.tensor.matmul(pa[:], wsum1_s[0:1, j * 128:(j + 1) * 128],
                             crow[:], start=False, stop=True)
            nc.scalar.activation(h_s[j][:, s0:s0 + SC], pa[:], AF.Gelu,
                                 bias=b1c_s[:, j:j + 1], scale=1.0)

    def emit_f2(sc):
        s0 = sc * SC
        for do in range(DT):
            po = ops_.tile([128, SC], F32, tag="po")
            for j in range(DT):
                nc.tensor.matmul(po[:], w2_s[j][:, do * 128:(do + 1) * 128],
                                 h_s[j][:, s0:s0 + SC],
                                 start=(j == 0), stop=(j == DT - 1))
            o = opool.tile([128, SC], F32, tag="o")
            nc.scalar.activation(o[:], po[:], AF.Identity,
                                 bias=b2c_s[:, do:do + 1], scale=1.0)
            nc.gpsimd.dma_start(outT[do * 128:(do + 1) * 128, s0:s0 + SC], o[:])

    # Interleaved schedule: stats(sc+1) is emitted right after F1(sc), and
    # F2(sc) sits between stats(sc+1) and F1(sc+1), so the ~3us DVE/ACT LN
    # chain (rows -> rsqrt -> broadcast -> v) hides under PE's F2 segment.
    usq0 = emit_prep(0)
    c0 = emit_stats(0, usq0)
    usq1 = emit_prep(1)
    emit_f1(0, c0)
    c1 = emit_stats(1, usq1)
    usq2 = emit_prep(2)
    emit_f2(0)
    emit_f1(1, c1)
    c2 = emit_stats(2, usq2)
    usq3 = emit_prep(3)
    emit_f2(1)
    emit_f1(2, c2)
    c3 = emit_stats(3, usq3)
    emit_f2(2)
    emit_f1(3, c3)
    emit_f2(3)


_NC_CACHE = {}


def _build_nc():
    if "nc" in _NC_CACHE:
        return _NC_CACHE["nc"]
    nc = bacc.Bacc("TRN2", target_bir_lowering=False, debug=False)
    xT = nc.declare_dram_parameter("xT", [D, S], F32, isOutput=False)
    wfe = nc.declare_dram_parameter("wfe", [KE * 128, D], BF16, isOutput=False)
    wfo = nc.declare_dram_parameter("wfo", [KO * 128, D], BF16, isOutput=False)
    ce = nc.declare_dram_parameter("ce", [KE * 128, CEC], BF16, isOutput=False)
    co = nc.declare_dram_parameter("co", [KO * 128, CEC], BF16, isOutput=False)
    w1b = nc.declare_dram_parameter("w1b", [D, D], BF16, isOutput=False)
    w2b = nc.declare_dram_parameter("w2b", [D, D], BF16, isOutput=False)
    wsum1r = nc.declare_dram_parameter("wsum1r", [1, D], BF16, isOutput=False)
    b1c = nc.declare_dram_parameter("b1c", [128, DT], F32, isOutput=False)
    b2c = nc.declare_dram_parameter("b2c", [128, DT], F32, isOutput=False)
    onesb = nc.declare_dram_parameter("onesb", [128, 1], BF16, isOutput=False)
    onesD = nc.declare_dram_parameter("onesD", [1, 128], F32R, isOutput=False)
    outT = nc.declare_dram_parameter("outT", [D, S], F32, isOutput=True)
    with tile.TileContext(nc) as tc:
        with ExitStack() as ctx:
            _emit_kernel(ctx, tc, xT, wfe, wfo, ce, co, w1b, w2b, wsum1r,
                         b1c, b2c, onesb, onesD, outT)
    nc.compile()
    _NC_CACHE["nc"] = nc
    return nc


def _host_prep(x, ln_g, ln_b, w1, b1, w2, b2):
    """Build per-core and shared device inputs."""
    B = x.shape[0]
    p_e = np.arange(513, dtype=np.float64)
    p_o = np.arange(512, dtype=np.float64)
    ss = np.arange(513, dtype=np.float64)
    ce_ = np.zeros((KE * 128, CEC), BF)
    ce_[:513, :513] = np.cos(2.0 * np.pi * np.outer(p_e, ss) / 1024.0).astype(BF)
    co_ = np.zeros((KO * 128, CEC), BF)
    co_[:512, :513] = np.cos(
        np.pi * np.outer(2.0 * p_o + 1.0, ss) / 1024.0).astype(BF)

    w1p = (w1 * ln_g[:, None]).astype(np.float32)
    w1pb = w1p.astype(BF)
    w2b_ = np.asarray(w2, np.float32).astype(BF)
    wsum1 = w1pb.astype(np.float64).sum(axis=0).astype(BF).reshape(1, D)
    b1p = (b1 + ln_b @ w1).astype(np.float32)
    b1c_ = np.ascontiguousarray(b1p.reshape(DT, 128).T)
    b2c_ = np.ascontiguousarray(np.asarray(b2, np.float32).reshape(DT, 128).T)

    rev = np.concatenate([[0], np.arange(D - 1, 0, -1)])
    shared = dict(ce=ce_, co=co_, w1b=w1pb, w2b=w2b_, wsum1r=wsum1,
                  b1c=b1c_, b2c=b2c_,
                  onesb=np.ones((128, 1), BF),
                  onesD=np.full((1, 128), float(D), np.float32))

    in_maps = []
    for b in range(B):
        xb = np.asarray(x[b], np.float32)
        w = np.float32(D) * xb[:, rev]
        wf_ = np.zeros((1025, D), np.float32)
        wf_[0] = w[0]
        wf_[1024] = w[1024]
        wf_[1:1024] = w[1:1024] + w[2047:1024:-1]
        wfe_ = np.zeros((KE * 128, D), BF)
        wfe_[:513] = wf_[0::2].astype(BF)
        wfo_ = np.zeros((KO * 128, D), BF)
        wfo_[:512] = wf_[1::2].astype(BF)
        xT = np.ascontiguousarray(xb.T)
        in_maps.append(dict(xT=xT, wfe=wfe_, wfo=wfo_, **shared))
    return in_maps


def _run(inputs, trace=False, trace_kwargs=None):
    x = np.asarray(inputs["x"], np.float32)
    in_maps = _host_prep(
        x,
        np.asarray(inputs["ln_g"], np.float32),
        np.asarray(inputs["ln_b"], np.float32),
        np.asarray(inputs["w1"], np.float32),
        np.asarray(inputs["b1"], np.float32),
        np.asarray(inputs["w2"], np.float32),
        np.asarray(inputs["b2"], np.float32),
    )
    nc = _build_nc()
    res = run_bass_kernel_spmd(nc, in_maps, list(range(NCORES)), trace=trace,
                               **(trace_kwargs or {}))
    outs = np.stack([np.ascontiguousarray(
        np.asarray(res.results[b]["outT"], np.float32).T)
        for b in range(NCORES)])
    outs = outs * np.asarray(inputs["mask"], np.float32)
    return outs, res


def kernel(**inputs) -> np.ndarray:
    out, _ = _run(inputs, trace=False)
    return out
